# revision 16
# baseline (speedup 1.0000x reference)
"""Trainium2 Bass kernel for nn_Cross_head (sparse_attention patch-correction).

Math (non-overlapping unfold/fold are inverse permutations):
    y   = W @ x + b                       (1x1x1 conv over channels)
    out = leaky_relu(y * (y + foldA + 1), 0.2)
    foldA = fold(attentions / (count_nonzero(attentions, -1) + 1e-5))

Sharding: spatial, across the 576 patch-columns (72 per core).  Every DMA
transfer is contiguous on the DRAM side; the patch<->voxel permutation rides
on (a) the matmul's strided rhs access pattern (stream x columns in patch
order so PSUM lands in patch layout) and (b) the final activation's strided
write (back to voxel order).
"""

import os
import sys

import numpy as np

sys.path.insert(0, "/opt/trn_rl_repo")

# ---- geometry (hardcoded for this problem) ----
C = 128          # channels (in == out)
D = 36           # depth
HWFULL = 5184    # H*W = 72*72
PS = 9           # patch size
NDP = 4          # D // PS
NWP = 576        # HWFULL // PS  (patch columns)
NCORES = 8
IWG = NWP // NCORES   # 72 patch columns per core
HWL = IWG * PS        # 648 voxel columns per core
NSUB = 2              # split each iD block into halves along iW
IWT = IWG // NSUB     # 36 patch columns per subtile
FT = IWT * 81         # 2916 free elements per subtile (patch layout)
HWT = IWT * PS        # 324 voxel columns per subtile
CHW = 6               # patch columns per matmul chunk
NMM = IWT // CHW      # 6 matmuls per subtile
NGRP = 2              # psum groups per subtile
MMG = NMM // NGRP     # 3 matmuls per psum group
MMN = CHW * 81        # 486 matmul free dim
BANK = 512            # fp32 elements per PSUM bank

_NC_CACHE = {}
LAST_RESULT = None


def _build_nc(use_lrelu_act=True, mm_dtype="float32r"):
    from contextlib import ExitStack

    import concourse.bacc as bacc
    import concourse.tile as tile
    from concourse import mybir

    f32 = mybir.dt.float32
    bf16 = mybir.dt.bfloat16
    AL = mybir.AluOpType
    AF = mybir.ActivationFunctionType

    nc = bacc.Bacc(
        "TRN2",
        target_bir_lowering=False,
        debug=False,
        enable_asserts=False,
        num_devices=NCORES,
    )
    x_d = nc.dram_tensor("x", [C, D, HWL], f32, kind="ExternalInput").ap()
    a_d = nc.dram_tensor("att", [C, NDP, IWG * 81], f32, kind="ExternalInput").ap()
    wt_d = nc.dram_tensor("wt", [C, C], f32, kind="ExternalInput").ap()
    b_d = nc.dram_tensor("bias", [C, 2], f32, kind="ExternalInput").ap()
    o_d = nc.dram_tensor("out", [C, D, HWL], f32, kind="ExternalOutput").ap()

    mmdt = f32 if mm_dtype == "float32" else getattr(mybir.dt, mm_dtype)
    mm_cast = mm_dtype != "float32"

    with tile.TileContext(nc) as tc, ExitStack() as ctx:
        const = ctx.enter_context(tc.tile_pool(name="const", bufs=1))
        wt_sb = const.tile([C, C], mmdt)
        if mm_cast:
            nc.gpsimd.dma_start(wt_sb[:], wt_d[:])  # casts during DMA
        else:
            nc.sync.dma_start(wt_sb[:], wt_d[:])
        b_sb = const.tile([C, 2], f32)
        nc.sync.dma_start(b_sb[:], b_d[:])
        b_ap = b_sb[:, 0:1]
        bp1_ap = b_sb[:, 1:2]
        alpha_sb = const.tile([C, 1], f32)
        nc.vector.memset(alpha_sb[:], 0.2)

        xp = ctx.enter_context(tc.tile_pool(name="xp", bufs=3))
        atp = ctx.enter_context(tc.tile_pool(name="atp", bufs=3))
        sgp = ctx.enter_context(tc.tile_pool(name="sgp", bufs=2))
        nzp = ctx.enter_context(tc.tile_pool(name="nzp", bufs=2))
        Apl = ctx.enter_context(tc.tile_pool(name="Apl", bufs=2))
        tpl = ctx.enter_context(tc.tile_pool(name="tpl", bufs=2))
        prp = ctx.enter_context(tc.tile_pool(name="prp", bufs=2))
        ovp = ctx.enter_context(tc.tile_pool(name="ovp", bufs=3))
        psp = ctx.enter_context(tc.tile_pool(name="psp", bufs=2, space="PSUM"))

        for iD in range(NDP):
            for h in range(NSUB):
                # ---- contiguous loads ----
                xt = xp.tile([C, PS * HWT], mmdt)
                xsrc = x_d[:, iD * PS : (iD + 1) * PS, h * HWT : (h + 1) * HWT]
                if mm_cast:
                    nc.gpsimd.dma_start(xt[:], xsrc)  # casts during DMA
                else:
                    nc.sync.dma_start(xt[:], xsrc)
                at = atp.tile([C, FT], f32)
                nc.sync.dma_start(at[:], a_d[:, iD, h * FT : (h + 1) * FT])

                # ---- nz = count_nonzero per patch:  sum |sign(att)| ----
                st = sgp.tile([C, FT], bf16)
                nc.scalar.activation(st[:], at[:], AF.Sign)
                nzv = nzp.tile([C, IWT], f32)
                nc.vector.tensor_reduce(
                    nzv[:],
                    st[:].rearrange("c (w k) -> c w k", k=81),
                    mybir.AxisListType.X,
                    AL.add,
                    apply_absolute_value=True,
                )
                nzr = nzp.tile([C, IWT], f32)
                nc.vector.tensor_scalar_add(nzv[:], nzv[:], 1e-5)
                nc.vector.reciprocal_approx_fast(nzr[:], nzv[:])

                # ---- A = att * (1/nz)  (broadcast over the 81 patch slots) ----
                # Written in the same column order the matmul streams, so the
                # scalar_tensor_tensor ops line up with PSUM contiguously.
                At = Apl.tile([C, FT], f32)
                if mm_cast:
                    # fp32r requires an even innermost count on the moving
                    # operand; stream chunk columns as (p1, p2, iW_loc:6).
                    in4 = at[:].rearrange(
                        "c (ch iw p q) -> c ch iw p q", ch=NMM, iw=CHW, q=PS
                    )
                    out4 = (
                        At[:]
                        .rearrange("c (ch p q iw) -> c ch p q iw", ch=NMM, q=PS, iw=CHW)
                        .transpose([0, 1, 4, 2, 3])
                    )
                    nzr4 = (
                        nzr[:]
                        .rearrange("c (ch iw) -> c ch iw", ch=NMM)
                        .unsqueeze(3)
                        .unsqueeze(4)
                        .broadcast_to((C, NMM, CHW, PS, PS))
                    )
                    nc.vector.tensor_tensor(out4, in4, nzr4, AL.mult)
                else:
                    nc.vector.tensor_tensor(
                        At[:].rearrange("c (w k) -> c w k", k=81),
                        at[:].rearrange("c (w k) -> c w k", k=81),
                        nzr[:].unsqueeze(2).broadcast_to((C, IWT, 81)),
                        AL.mult,
                    )

                # ---- GEMM: psum = W @ x, columns streamed in patch order ----
                # xt free layout: (p1:9, iW:36, p2:9)
                x4 = xt[:].rearrange("c (a b d) -> c a b d", a=PS, d=PS)
                if mm_cast:
                    x4 = x4.transpose([0, 1, 3, 2])  # [C, p1:9, p2:9, iW:36]
                else:
                    x4 = x4.transpose([0, 2, 1, 3])  # [C, iW:36, p1:9, p2:9]
                pst = []
                for g in range(NGRP):
                    ps_t = psp.tile([C, MMG * BANK], f32)  # 3 banks
                    pst.append(ps_t)
                    for m in range(MMG):
                        ch = g * MMG + m
                        if mm_cast:
                            rhs = x4[:, :, :, ch * CHW : (ch + 1) * CHW]
                        else:
                            rhs = x4[:, ch * CHW : (ch + 1) * CHW, :, :]
                        nc.tensor.matmul(
                            ps_t[:, m * BANK : m * BANK + MMN],
                            wt_sb[:],
                            rhs,
                            start=True,
                            stop=True,
                        )

                # ---- t = (A + (b+1)) + psum ; pre = (psum + b) * t ----
                tt = tpl.tile([C, FT], f32)
                pre = prp.tile([C, FT], f32)
                for g in range(NGRP):
                    ps_ap = (
                        pst[g][:]
                        .rearrange("c (m n) -> c m n", n=BANK)[:, :, 0:MMN]
                    )  # [C, 3, 486]
                    sl = slice(g * MMG * MMN, (g + 1) * MMG * MMN)
                    A3 = At[:, sl].rearrange("c (m n) -> c m n", n=MMN)
                    t3 = tt[:, sl].rearrange("c (m n) -> c m n", n=MMN)
                    nc.vector.scalar_tensor_tensor(
                        t3, A3, bp1_ap, ps_ap, AL.add, AL.add
                    )
                    p3 = pre[:, sl].rearrange("c (m n) -> c m n", n=MMN)
                    nc.vector.scalar_tensor_tensor(
                        p3, ps_ap, b_ap, t3, AL.add, AL.mult
                    )

                # ---- out = lrelu(pre), written back in voxel order ----
                ov = ovp.tile([C, PS * HWT], f32)
                if mm_cast:
                    # pre is chunked (ch, p1, p2, iw); un-permute per chunk
                    # (ACT mem pattern allows at most 3 free dims).
                    ov4 = (
                        ov[:]
                        .rearrange("c (p hw) -> c p hw", p=PS)
                    )
                    for ch in range(NMM):
                        pre_c = pre[
                            :, ch * MMN : (ch + 1) * MMN
                        ].rearrange("c (p q iw) -> c p q iw", p=PS, iw=CHW)
                        # voxel pos = p*HWT + (ch*CHW+iw)*PS + q
                        ov_c = (
                            ov[:]
                            .rearrange(
                                "c (p ww q) -> c p ww q", p=PS, q=PS
                            )[:, :, ch * CHW : (ch + 1) * CHW, :]
                            .transpose([0, 1, 3, 2])
                        )  # dims (p, q, iw) steps (HWT, 1, PS)
                        nc.scalar.activation(
                            ov_c, pre_c, AF.Prelu, alpha=alpha_sb[:, 0:1]
                        )
                else:
                    ov_ap = (
                        ov[:]
                        .rearrange("c (a b d) -> c a b d", a=PS, d=PS)
                        .transpose([0, 2, 1, 3])
                    )  # [C, 36, 9, 9] steps (9, HWT, 1)
                    pre_ap = pre[:].rearrange("c (w p q) -> c w p q", p=PS, q=PS)
                    if use_lrelu_act:
                        nc.scalar.activation(
                            ov_ap, pre_ap, AF.Prelu, alpha=alpha_sb[:, 0:1]
                        )
                    else:
                        nc.vector.scalar_tensor_tensor(
                            ov_ap, pre_ap, 0.2, pre_ap, AL.mult, AL.max
                        )

                # ---- contiguous store ----
                nc.sync.dma_start(
                    o_d[:, iD * PS : (iD + 1) * PS, h * HWT : (h + 1) * HWT], ov[:]
                )

    nc.compile()
    return nc


def _get_nc(**kw):
    key = tuple(sorted(kw.items()))
    if key not in _NC_CACHE:
        _NC_CACHE[key] = _build_nc(**kw)
    return _NC_CACHE[key]


def kernel(x, attentions, W, b, **build_kw):
    global LAST_RESULT
    from concourse.bass_utils import run_bass_kernel_spmd

    x = np.asarray(x, dtype=np.float32)
    attentions = np.asarray(attentions, dtype=np.float32)
    W = np.asarray(W, dtype=np.float32)
    b = np.asarray(b, dtype=np.float32)

    nc = _get_nc(**build_kw)

    xs = x.reshape(C, D, NCORES, HWL)
    as4 = attentions.reshape(C, NDP, NCORES, IWG, 81)
    wt = np.ascontiguousarray(W.T)
    bcol = np.ascontiguousarray(np.stack([b, b + 1.0], axis=1))

    in_maps = []
    for s in range(NCORES):
        in_maps.append(
            {
                "x": np.ascontiguousarray(xs[:, :, s, :]),
                "att": np.ascontiguousarray(as4[:, :, s, :, :]).reshape(
                    C, NDP, IWG * 81
                ),
                "wt": wt,
                "bias": bcol,
            }
        )

    res = run_bass_kernel_spmd(
        nc,
        in_maps,
        core_ids=list(range(NCORES)),
        trace=bool(os.environ.get("BASS_TRACE")),
    )
    LAST_RESULT = res

    out = np.empty((C, D, NCORES, HWL), dtype=np.float32)
    for s in range(NCORES):
        out[:, :, s, :] = res.results[s]["out"]
    return out.reshape(1, C, D, HWFULL)


# revision 17
# speedup vs baseline: 1.0496x; 1.0496x over previous
"""Trainium2 Bass kernel for nn_Cross_head (sparse_attention patch-correction).

Math (non-overlapping unfold/fold are inverse permutations, so
corr = y + fold(attentions/nz)):
    y   = W @ x + b                       (1x1x1 conv over channels)
    out = leaky_relu(y * (y + foldA + 1), 0.2)
    foldA = fold(attentions / (count_nonzero(attentions, -1) + 1e-5))

Sharding: spatial, across the 576 patch-columns (72 per core); no
cross-core communication.  Every DMA transfer is contiguous on the DRAM
side.  Per subtile (9 d-rows x 36 patch columns) the matmul streams x's
columns one d-row at a time in (p2, iW) order (innermost count 36, even,
which float32r requires), so PSUM holds y in (p1, p2, iW) order; the
patch<->psum<->voxel permutations ride on strided access patterns of ops
we need anyway (the att*1/nz multiply and the final leaky-relu).

Engine assignment per subtile:
  sync   : att load, out store (HWDGE)
  gpsimd : x load with fp32->fp32r cast (SWDGE), W load+cast
  tensor : 9 matmuls (f32r, full-rate) into 3 psum groups
  scalar : Sign(att) for the nonzero count; final Prelu(alpha=.2)
  vector : segmented reduce -> 1/nz, A = att*r, t = (A+(b+1))+psum,
           pre = (psum+b)*t
"""

import os
import sys

import numpy as np

sys.path.insert(0, "/opt/trn_rl_repo")

# ---- geometry (hardcoded for this problem) ----
C = 128          # channels (in == out)
D = 36           # depth
HWFULL = 5184    # H*W = 72*72
PS = 9           # patch size
NDP = 4          # D // PS
NWP = 576        # HWFULL // PS  (patch columns)
NCORES = 8
IWG = NWP // NCORES   # 72 patch columns per core
HWL = IWG * PS        # 648 voxel columns per core
NSUB = 2              # split each iD block into halves along iW
IWT = IWG // NSUB     # 36 patch columns per subtile
FT = IWT * 81         # 2916 free elements per subtile
HWT = IWT * PS        # 324 voxel columns per subtile
NMM = PS              # 9 matmuls per subtile (one per d-row)
MMN = PS * IWT        # 324 matmul free dim: (p2, iW) for one d-row
NGRP = 3              # psum groups per subtile
MMG = NMM // NGRP     # 3 matmuls (d-rows) per psum group
BANK = 512            # fp32 elements per PSUM bank

_NC_CACHE = {}
LAST_RESULT = None


def _build_nc(mm_dtype="float32r", amul_engine="vector"):
    from contextlib import ExitStack

    import concourse.bacc as bacc
    import concourse.tile as tile
    from concourse import mybir

    f32 = mybir.dt.float32
    bf16 = mybir.dt.bfloat16
    AL = mybir.AluOpType
    AF = mybir.ActivationFunctionType

    nc = bacc.Bacc(
        "TRN2",
        target_bir_lowering=False,
        debug=False,
        enable_asserts=False,
        num_devices=NCORES,
    )
    x_d = nc.dram_tensor("x", [C, D, HWL], f32, kind="ExternalInput").ap()
    a_d = nc.dram_tensor("att", [C, NDP, IWG * 81], f32, kind="ExternalInput").ap()
    wt_d = nc.dram_tensor("wt", [C, C], f32, kind="ExternalInput").ap()
    b_d = nc.dram_tensor("bias", [C, 2], f32, kind="ExternalInput").ap()
    o_d = nc.dram_tensor("out", [C, D, HWL], f32, kind="ExternalOutput").ap()

    mmdt = f32 if mm_dtype == "float32" else getattr(mybir.dt, mm_dtype)
    mm_cast = mm_dtype != "float32"

    with tile.TileContext(nc) as tc, ExitStack() as ctx:
        const = ctx.enter_context(tc.tile_pool(name="const", bufs=1))
        wt_sb = const.tile([C, C], mmdt)
        if mm_cast:
            nc.gpsimd.dma_start(wt_sb[:], wt_d[:])  # casts during DMA
        else:
            nc.sync.dma_start(wt_sb[:], wt_d[:])
        b_sb = const.tile([C, 2], f32)
        nc.sync.dma_start(b_sb[:], b_d[:])
        b_ap = b_sb[:, 0:1]
        bp1_ap = b_sb[:, 1:2]
        alpha_sb = const.tile([C, 1], f32)
        nc.vector.memset(alpha_sb[:], 0.2)

        xp = ctx.enter_context(tc.tile_pool(name="xp", bufs=3))
        atp = ctx.enter_context(tc.tile_pool(name="atp", bufs=3))
        sgp = ctx.enter_context(tc.tile_pool(name="sgp", bufs=2))
        nzp = ctx.enter_context(tc.tile_pool(name="nzp", bufs=2))
        Apl = ctx.enter_context(tc.tile_pool(name="Apl", bufs=2))
        tpl = ctx.enter_context(tc.tile_pool(name="tpl", bufs=2))
        prp = ctx.enter_context(tc.tile_pool(name="prp", bufs=2))
        ovp = ctx.enter_context(tc.tile_pool(name="ovp", bufs=3))
        psp = ctx.enter_context(tc.tile_pool(name="psp", bufs=2, space="PSUM"))

        amul = nc.gpsimd if amul_engine == "gpsimd" else nc.vector

        for iD in range(NDP):
            for h in range(NSUB):
                # ---- contiguous loads ----
                xt = xp.tile([C, PS * HWT], mmdt)
                xsrc = x_d[:, iD * PS : (iD + 1) * PS, h * HWT : (h + 1) * HWT]
                if mm_cast:
                    nc.gpsimd.dma_start(xt[:], xsrc)  # casts during DMA
                else:
                    nc.sync.dma_start(xt[:], xsrc)
                at = atp.tile([C, FT], f32)
                nc.sync.dma_start(at[:], a_d[:, iD, h * FT : (h + 1) * FT])

                # ---- nz = count_nonzero per patch:  sum |sign(att)| ----
                st = sgp.tile([C, FT], bf16)
                nc.scalar.activation(st[:], at[:], AF.Sign)
                nzv = nzp.tile([C, IWT], f32)
                nc.vector.tensor_reduce(
                    nzv[:],
                    st[:].rearrange("c (w k) -> c w k", k=81),
                    mybir.AxisListType.X,
                    AL.add,
                    apply_absolute_value=True,
                )
                nzr = nzp.tile([C, IWT], f32)
                nc.vector.tensor_scalar_add(nzv[:], nzv[:], 1e-5)
                nc.vector.reciprocal_approx_fast(nzr[:], nzv[:])

                # ---- A = att * (1/nz), written in psum order (p1, p2, iW) --
                # at free layout is (iW:36, p1:9, p2:9); read it strided in
                # (p1, p2, iW) order, write contiguous.
                At = Apl.tile([C, FT], f32)
                a3 = (
                    at[:]
                    .rearrange("c (iw p q) -> c iw p q", p=PS, q=PS)
                    .transpose([0, 2, 3, 1])
                )  # dims (p1, p2, iW) steps (9, 1, 81)
                nzr3 = (
                    nzr[:]
                    .unsqueeze(1)
                    .unsqueeze(2)
                    .broadcast_to((C, PS, PS, IWT))
                )
                A3 = At[:].rearrange("c (p q iw) -> c p q iw", p=PS, q=PS)
                amul.tensor_tensor(A3, a3, nzr3, AL.mult)

                # ---- GEMM: psum = W @ x, one matmul per d-row ----
                # xt free layout: (p1:9, hw:324) with hw = iW*PS + p2;
                # stream (p2, iW): innermost count 36 (even, f32r-legal)
                x4 = (
                    xt[:]
                    .rearrange("c (p iw q) -> c p iw q", p=PS, q=PS)
                    .transpose([0, 1, 3, 2])
                )  # [C, p1:9, p2:9, iW:36]
                pst = []
                for g in range(NGRP):
                    ps_t = psp.tile([C, MMG * BANK], f32)  # 3 banks
                    pst.append(ps_t)
                    for m in range(MMG):
                        dr = g * MMG + m  # d-row
                        nc.tensor.matmul(
                            ps_t[:, m * BANK : m * BANK + MMN],
                            wt_sb[:],
                            x4[:, dr, :, :],
                            start=True,
                            stop=True,
                        )

                # ---- t = (A + (b+1)) + psum ; pre = (psum + b) * t ----
                tt = tpl.tile([C, FT], f32)
                pre = prp.tile([C, FT], f32)
                for g in range(NGRP):
                    ps_ap = (
                        pst[g][:]
                        .rearrange("c (m n) -> c m n", n=BANK)[:, :, 0:MMN]
                    )  # [C, 3, 324]
                    sl = slice(g * MMG * MMN, (g + 1) * MMG * MMN)
                    A2 = At[:, sl].rearrange("c (m n) -> c m n", n=MMN)
                    t2 = tt[:, sl].rearrange("c (m n) -> c m n", n=MMN)
                    p2_ = pre[:, sl].rearrange("c (m n) -> c m n", n=MMN)
                    nc.vector.scalar_tensor_tensor(
                        t2, A2, bp1_ap, ps_ap, AL.add, AL.add
                    )
                    nc.vector.scalar_tensor_tensor(
                        p2_, ps_ap, b_ap, t2, AL.add, AL.mult
                    )

                # ---- out = lrelu(pre), un-permute (p1,p2,iW) -> voxel ----
                ov = ovp.tile([C, PS * HWT], f32)
                pre_ap = pre[:].rearrange(
                    "c (p q iw) -> c p q iw", p=PS, q=PS
                )  # contiguous (p1, p2, iW)
                ov_ap = (
                    ov[:]
                    .rearrange("c (p iw q) -> c p iw q", p=PS, q=PS)
                    .transpose([0, 1, 3, 2])
                )  # dims (p1, p2, iW) steps (HWT, 1, PS)
                nc.scalar.activation(
                    ov_ap, pre_ap, AF.Prelu, alpha=alpha_sb[:, 0:1]
                )

                # ---- contiguous store ----
                nc.sync.dma_start(
                    o_d[:, iD * PS : (iD + 1) * PS, h * HWT : (h + 1) * HWT], ov[:]
                )

    nc.compile()
    return nc


def _get_nc(**kw):
    key = tuple(sorted(kw.items()))
    if key not in _NC_CACHE:
        _NC_CACHE[key] = _build_nc(**kw)
    return _NC_CACHE[key]


def kernel(x, attentions, W, b, **build_kw):
    global LAST_RESULT
    from concourse.bass_utils import run_bass_kernel_spmd

    x = np.asarray(x, dtype=np.float32)
    attentions = np.asarray(attentions, dtype=np.float32)
    W = np.asarray(W, dtype=np.float32)
    b = np.asarray(b, dtype=np.float32)

    nc = _get_nc(**build_kw)

    xs = x.reshape(C, D, NCORES, HWL)
    as4 = attentions.reshape(C, NDP, NCORES, IWG, 81)
    wt = np.ascontiguousarray(W.T)
    bcol = np.ascontiguousarray(np.stack([b, b + 1.0], axis=1))

    in_maps = []
    for s in range(NCORES):
        in_maps.append(
            {
                "x": np.ascontiguousarray(xs[:, :, s, :]),
                "att": np.ascontiguousarray(as4[:, :, s, :, :]).reshape(
                    C, NDP, IWG * 81
                ),
                "wt": wt,
                "bias": bcol,
            }
        )

    res = run_bass_kernel_spmd(
        nc,
        in_maps,
        core_ids=list(range(NCORES)),
        trace=bool(os.environ.get("BASS_TRACE")),
    )
    LAST_RESULT = res

    out = np.empty((C, D, NCORES, HWL), dtype=np.float32)
    for s in range(NCORES):
        out[:, :, s, :] = res.results[s]["out"]
    return out.reshape(1, C, D, HWFULL)


# revision 20
# speedup vs baseline: 1.2137x; 1.1564x over previous
"""Trainium2 Bass kernel for nn_Cross_head (sparse_attention patch-correction).

Math (non-overlapping unfold/fold are inverse permutations, so
corr = y + fold(attentions/nz)):
    y   = W @ x + b                       (1x1x1 conv over channels)
    out = leaky_relu(y * (y + foldA + 1), 0.2)
    foldA = fold(attentions / (count_nonzero(attentions, -1) + 1e-5))

Sharding: spatial, across the 576 patch-columns (72 per core); no
cross-core communication.  Every DMA transfer is contiguous on the DRAM
side.  Per subtile (9 d-rows x 36 patch columns) the matmul streams x's
columns one d-row at a time in (p2, iW) order (innermost count 36, even,
which float32r requires), so PSUM holds y in (p1, p2, iW) order; the
patch<->psum<->voxel permutations ride on strided access patterns of ops
we need anyway (the att*1/nz multiply and the final leaky-relu).

Engine assignment per subtile:
  sync   : att load, out store (HWDGE)
  gpsimd : x load with fp32->fp32r cast (SWDGE), W load+cast
  tensor : 9 matmuls (f32r, full-rate) into 3 psum groups
  scalar : Sign(att) for the nonzero count; final Prelu(alpha=.2)
  vector : segmented reduce -> 1/nz, A = att*r, t = (A+(b+1))+psum,
           pre = (psum+b)*t
"""

import os
import sys

import numpy as np

sys.path.insert(0, "/opt/trn_rl_repo")

# ---- geometry (hardcoded for this problem) ----
C = 128          # channels (in == out)
D = 36           # depth
HWFULL = 5184    # H*W = 72*72
PS = 9           # patch size
NDP = 4          # D // PS
NWP = 576        # HWFULL // PS  (patch columns)
NCORES = 8
IWG = NWP // NCORES   # 72 patch columns per core
HWL = IWG * PS        # 648 voxel columns per core
NSUB = 2              # split each iD block into halves along iW
IWT = IWG // NSUB     # 36 patch columns per subtile
FT = IWT * 81         # 2916 free elements per subtile
HWT = IWT * PS        # 324 voxel columns per subtile
MMN = 486             # matmul free dim: arbitrary contiguous voxel slice
NMM = PS * HWT // MMN # 6 matmuls per subtile
NGRP = 2              # psum groups per subtile
MMG = NMM // NGRP     # 3 matmuls per psum group
BANK = 512            # fp32 elements per PSUM bank

_NC_CACHE = {}
LAST_RESULT = None


def _build_nc(mm_dtype="float32r", amul_engine="vector"):
    from contextlib import ExitStack

    import concourse.bacc as bacc
    import concourse.tile as tile
    from concourse import mybir

    f32 = mybir.dt.float32
    bf16 = mybir.dt.bfloat16
    AL = mybir.AluOpType
    AF = mybir.ActivationFunctionType

    nc = bacc.Bacc(
        "TRN2",
        target_bir_lowering=False,
        debug=False,
        enable_asserts=False,
        num_devices=NCORES,
    )
    x_d = nc.dram_tensor("x", [C, D, HWL], f32, kind="ExternalInput").ap()
    a_d = nc.dram_tensor("att", [C, NDP, IWG * 81], f32, kind="ExternalInput").ap()
    wt_d = nc.dram_tensor("wt", [C, C], f32, kind="ExternalInput").ap()
    b_d = nc.dram_tensor("bias", [C, 2], f32, kind="ExternalInput").ap()
    o_d = nc.dram_tensor("out", [C, D, HWL], f32, kind="ExternalOutput").ap()

    mmdt = f32 if mm_dtype == "float32" else getattr(mybir.dt, mm_dtype)
    mm_cast = mm_dtype != "float32"

    with tile.TileContext(nc) as tc, ExitStack() as ctx:
        const = ctx.enter_context(tc.tile_pool(name="const", bufs=1))
        wt_sb = const.tile([C, C], mmdt)
        if mm_cast:
            nc.gpsimd.dma_start(wt_sb[:], wt_d[:])  # casts during DMA
        else:
            nc.sync.dma_start(wt_sb[:], wt_d[:])
        b_sb = const.tile([C, 2], f32)
        nc.sync.dma_start(b_sb[:], b_d[:])
        b_ap = b_sb[:, 0:1]
        bp1_ap = b_sb[:, 1:2]
        alpha_sb = const.tile([C, 1], f32)
        nc.vector.memset(alpha_sb[:], 0.2)

        xp = ctx.enter_context(tc.tile_pool(name="xp", bufs=3))
        atp = ctx.enter_context(tc.tile_pool(name="atp", bufs=3))
        sgp = ctx.enter_context(tc.tile_pool(name="sgp", bufs=2))
        nzp = ctx.enter_context(tc.tile_pool(name="nzp", bufs=2))
        Apl = ctx.enter_context(tc.tile_pool(name="Apl", bufs=2))
        tpl = ctx.enter_context(tc.tile_pool(name="tpl", bufs=2))
        prp = ctx.enter_context(tc.tile_pool(name="prp", bufs=2))
        ovp = ctx.enter_context(tc.tile_pool(name="ovp", bufs=3))
        psp = ctx.enter_context(tc.tile_pool(name="psp", bufs=2, space="PSUM"))

        amul = nc.gpsimd if amul_engine == "gpsimd" else nc.vector

        for iD in range(NDP):
            for h in range(NSUB):
                # ---- contiguous loads ----
                xt = xp.tile([C, PS * HWT], mmdt)
                xsrc = x_d[:, iD * PS : (iD + 1) * PS, h * HWT : (h + 1) * HWT]
                if mm_cast:
                    nc.gpsimd.dma_start(xt[:], xsrc)  # casts during DMA
                else:
                    nc.sync.dma_start(xt[:], xsrc)
                at = atp.tile([C, FT], f32)
                nc.sync.dma_start(at[:], a_d[:, iD, h * FT : (h + 1) * FT])

                # ---- nz = count_nonzero per patch:  sum |sign(att)| ----
                st = sgp.tile([C, FT], bf16)
                nc.scalar.activation(st[:], at[:], AF.Sign)
                nzv = nzp.tile([C, IWT], f32)
                nc.vector.tensor_reduce(
                    nzv[:],
                    st[:].rearrange("c (w k) -> c w k", k=81),
                    mybir.AxisListType.X,
                    AL.add,
                    apply_absolute_value=True,
                )
                nzr = nzp.tile([C, IWT], f32)
                nc.vector.tensor_scalar_add(nzv[:], nzv[:], 1e-5)
                nc.vector.reciprocal_approx_fast(nzr[:], nzv[:])

                # ---- A = att * (1/nz), written in VOXEL order (p1, iW, p2) -
                # at free layout is (iW:36, p1:9, p2:9); read it strided in
                # (p1, iW, p2) order (innermost contiguous runs of 9), write
                # contiguous.  Everything downstream is then voxel-ordered.
                At = Apl.tile([C, FT], f32)
                a3 = (
                    at[:]
                    .rearrange("c (iw p q) -> c iw p q", p=PS, q=PS)
                    .transpose([0, 2, 1, 3])
                )  # dims (p1, iW, p2) steps (9, 81, 1)
                nzr3 = (
                    nzr[:]
                    .unsqueeze(1)
                    .unsqueeze(3)
                    .broadcast_to((C, PS, IWT, PS))
                )
                A3 = At[:].rearrange("c (p iw q) -> c p iw q", p=PS, q=PS)
                amul.tensor_tensor(A3, a3, nzr3, AL.mult)

                # ---- GEMM: psum = W @ x, plain contiguous voxel slices ----
                pst = []
                for g in range(NGRP):
                    ps_t = psp.tile([C, MMG * BANK], f32)  # 3 banks
                    pst.append(ps_t)
                    for m in range(MMG):
                        ch = g * MMG + m
                        nc.tensor.matmul(
                            ps_t[:, m * BANK : m * BANK + MMN],
                            wt_sb[:],
                            xt[:, ch * MMN : (ch + 1) * MMN],
                            start=True,
                            stop=True,
                        )

                # ---- t = (A + (b+1)) + psum ; pre = (psum + b) * t ----
                tt = tpl.tile([C, FT], f32)
                pre = prp.tile([C, FT], f32)
                for g in range(NGRP):
                    ps_ap = (
                        pst[g][:]
                        .rearrange("c (m n) -> c m n", n=BANK)[:, :, 0:MMN]
                    )  # [C, 3, 324]
                    sl = slice(g * MMG * MMN, (g + 1) * MMG * MMN)
                    A2 = At[:, sl].rearrange("c (m n) -> c m n", n=MMN)
                    t2 = tt[:, sl].rearrange("c (m n) -> c m n", n=MMN)
                    p2_ = pre[:, sl].rearrange("c (m n) -> c m n", n=MMN)
                    nc.vector.scalar_tensor_tensor(
                        t2, A2, bp1_ap, ps_ap, AL.add, AL.add
                    )
                    nc.vector.scalar_tensor_tensor(
                        p2_, ps_ap, b_ap, t2, AL.add, AL.mult
                    )

                # ---- out = lrelu(pre); already voxel order, contiguous ----
                ov = ovp.tile([C, PS * HWT], f32)
                nc.scalar.activation(
                    ov[:], pre[:], AF.Prelu, alpha=alpha_sb[:, 0:1]
                )

                # ---- contiguous store ----
                nc.sync.dma_start(
                    o_d[:, iD * PS : (iD + 1) * PS, h * HWT : (h + 1) * HWT], ov[:]
                )

    nc.compile()
    return nc


def _get_nc(**kw):
    key = tuple(sorted(kw.items()))
    if key not in _NC_CACHE:
        _NC_CACHE[key] = _build_nc(**kw)
    return _NC_CACHE[key]


def kernel(x, attentions, W, b, **build_kw):
    global LAST_RESULT
    from concourse.bass_utils import run_bass_kernel_spmd

    x = np.asarray(x, dtype=np.float32)
    attentions = np.asarray(attentions, dtype=np.float32)
    W = np.asarray(W, dtype=np.float32)
    b = np.asarray(b, dtype=np.float32)

    nc = _get_nc(**build_kw)

    xs = x.reshape(C, D, NCORES, HWL)
    as4 = attentions.reshape(C, NDP, NCORES, IWG, 81)
    wt = np.ascontiguousarray(W.T)
    bcol = np.ascontiguousarray(np.stack([b, b + 1.0], axis=1))

    in_maps = []
    for s in range(NCORES):
        in_maps.append(
            {
                "x": np.ascontiguousarray(xs[:, :, s, :]),
                "att": np.ascontiguousarray(as4[:, :, s, :, :]).reshape(
                    C, NDP, IWG * 81
                ),
                "wt": wt,
                "bias": bcol,
            }
        )

    res = run_bass_kernel_spmd(
        nc,
        in_maps,
        core_ids=list(range(NCORES)),
        trace=bool(os.environ.get("BASS_TRACE")),
    )
    LAST_RESULT = res

    out = np.empty((C, D, NCORES, HWL), dtype=np.float32)
    for s in range(NCORES):
        out[:, :, s, :] = res.results[s]["out"]
    return out.reshape(1, C, D, HWFULL)
